# revision 2
# baseline (speedup 1.0000x reference)
"""MoE expert FFN (CachedKimiExperts) on 8 Trainium2 NeuronCores.

Expert-parallel sharding: core c owns experts [2c, 2c+1].  Routing
(softmax -> top-k -> renormalize) and token gather/scatter run on the
host; each core streams its two experts' weights (pre-transposed,
cast to fp16 on the host) from HBM once and computes

    h   = x_g @ w1[e].T          (gate/up fused, contract over H)
    act = silu(h[:, :I]) * h[:, I:]
    y   = act @ w2[e].T          (contract over I)

for the gathered token block of each expert.  The host applies the
routing weights and scatter-adds the per-expert outputs.

Matmul layout: gathered tokens (transposed) are the stationary operand,
weights stream through the PE at N=512; weight tiles are 1 MiB (two
128-row k-chunks paired in the free dim, prepared host-side) so DMA
descriptor-issue on SyncE stays well ahead of the 16 DMA engines.
"""

import os
import sys

import numpy as np

for _p in ("/opt/trn_rl_repo", "/root/.axon_site/_ro/trn_rl_repo"):
    if os.path.isdir(_p) and _p not in sys.path:
        sys.path.append(_p)

import concourse.bass as bass  # noqa: F401  (bass must import before tile)
import concourse.mybir as mybir
import concourse.tile as tile
from concourse import bacc, bass_utils
from concourse.masks import make_identity

N_CORES = 8
E = 16
E_LOC = E // N_CORES  # experts per core
H = 2048  # hidden dim
I = 1024  # expert intermediate dim
I2 = 2 * I  # fused gate+up width
P = 128  # partitions
FD = 512  # matmul moving free dim (one fp32 PSUM bank)

F16 = mybir.dt.float16
F32 = mybir.dt.float32

TRACE = False
TRACE_CORES = None
LAST_RESULTS = None

_programs = {}


def _build_program(C):
    """Bass/Tile program for one core: E_LOC experts x (C tokens each)."""
    KC1 = H // P  # k-chunks for the gate/up matmul (contract over H)
    KC2 = I // P  # k-chunks for the down matmul (contract over I)
    G1 = KC1 // 2  # w1 tile groups (2 k-chunks per 1MiB tile)
    G2 = KC2 // 2  # w2 tile groups
    CB = C // P  # token blocks per expert
    NH = H // 2

    nc = bacc.Bacc(
        "TRN2", target_bir_lowering=False, debug=False, num_devices=N_CORES
    )
    # w1p[e, g, p, j, :] = w1[2c+e].T[(2g+j)*128 + p, :]   (gate/up fused)
    w1p = nc.dram_tensor("w1p", [E_LOC, G1, P, 2, I2], F16, kind="ExternalInput")
    # w2p[e, g, p, j, :] = w2[2c+e].T[(2g+j)*128 + p, :]
    w2p = nc.dram_tensor("w2p", [E_LOC, G2, P, 2, H], F16, kind="ExternalInput")
    # xg[e, p, kc, c] = x.T[kc*128 + p, tok_c(e)]  (gathered, padded)
    xg = nc.dram_tensor("xg", [E_LOC, P, KC1, C], F16, kind="ExternalInput")
    y = nc.dram_tensor("y", [E_LOC, C, H], F16, kind="ExternalOutput")

    with tile.TileContext(nc) as tc:
        with (
            tc.tile_pool(name="w1pool", bufs=10) as w1pool,
            tc.tile_pool(name="w2pool", bufs=8) as w2pool,
            tc.tile_pool(name="xp", bufs=2) as xp,
            tc.tile_pool(name="actp", bufs=2) as actp,
            tc.tile_pool(name="yp", bufs=4) as yp,
            tc.tile_pool(name="constp", bufs=1) as constp,
            tc.tile_pool(name="pgu", bufs=2, space="PSUM") as pgu,
            tc.tile_pool(name="py", bufs=1, space="PSUM") as py,
        ):
            ident = constp.tile([P, P], F16, name="ident")
            make_identity(nc, ident)

            # HAM warmup: ~4us of dummy matmuls during the initial DMA wait
            # flips the PE clock gate to 2.4GHz before the real stream
            # starts (idle/cold default is 1.2GHz, needs ~3.4us of activity).
            # Output region is scratch -- the first real accumulation's
            # start=True clears the bank.
            warm_ps = pgu.tile([P, I], F32, tag="gu", name="warm_ps")
            for _ in range(40):
                nc.tensor.matmul(
                    warm_ps[:, :P], ident, ident, start=True, stop=True
                )

            for e in range(E_LOC):
                for cb in range(CB):
                    # split halves: the first matmul only needs kc=0
                    xg_t = xp.tile([P, KC1, P], F16, tag="xg", name="xg_t")
                    hk = KC1 // 2
                    cs = slice(cb * P, (cb + 1) * P)
                    nc.sync.dma_start(xg_t[:, :hk, :], xg[e, :, :hk, cs])
                    nc.sync.dma_start(xg_t[:, hk:, :], xg[e, :, hk:, cs])

                    # ---- gate/up projection: h[c, i2] = x @ w1[e].T ----
                    gate_ps = pgu.tile([P, I], F32, tag="gu", name="gate_ps")
                    up_ps = pgu.tile([P, I], F32, tag="gu", name="up_ps")
                    for g in range(G1):
                        w1_t = w1pool.tile(
                            [P, 2, I2], F16, tag="w1", name="w1_t"
                        )
                        if e == 0 and cb == 0 and g == 0:
                            # split the very first tile so matmuls start
                            # after 512KB instead of 1MB
                            nc.sync.dma_start(w1_t[:, 0, :], w1p[e, g, :, 0, :])
                            nc.sync.dma_start(w1_t[:, 1, :], w1p[e, g, :, 1, :])
                        else:
                            nc.sync.dma_start(w1_t, w1p[e, g])
                        for j in range(2):
                            kc = 2 * g + j
                            lhsT = xg_t[:, kc, :]
                            st = kc == 0
                            sp = kc == KC1 - 1
                            for nb in range(I // FD):
                                nc.tensor.matmul(
                                    gate_ps[:, nb * FD : (nb + 1) * FD],
                                    lhsT,
                                    w1_t[:, j, nb * FD : (nb + 1) * FD],
                                    start=st,
                                    stop=sp,
                                )
                            for nb in range(I // FD):
                                nc.tensor.matmul(
                                    up_ps[:, nb * FD : (nb + 1) * FD],
                                    lhsT,
                                    w1_t[:, j, I + nb * FD : I + (nb + 1) * FD],
                                    start=st,
                                    stop=sp,
                                )

                    # ---- act = silu(gate) * up, cast to fp16 ----
                    # Chunked (4x 256 cols) so the first transpose / mm2
                    # matmuls start ~1.5us after mm1 ends -- keeps the PE
                    # inside the HAM warm window (idle >3.4us re-throttles
                    # the clock to 1.2GHz for the whole down-proj).
                    QW = I // 4
                    sg = actp.tile([P, I], F32, tag="sg", name="sg")
                    act = actp.tile([P, I], F16, tag="act", name="act")
                    for q in range(4):
                        qs = slice(q * QW, (q + 1) * QW)
                        nc.scalar.activation(
                            sg[:, qs],
                            gate_ps[:, qs],
                            mybir.ActivationFunctionType.Silu,
                        )
                        nc.vector.tensor_mul(act[:, qs], sg[:, qs], up_ps[:, qs])

                    # ---- transpose + down projection, interleaved per
                    # i-chunk: y[c, h] = act @ w2[e].T ----
                    # (tp shares the gu pool's slots; fp16 -> one PSUM bank)
                    tp_ps = pgu.tile([P, KC2, P], F16, tag="gu", name="tp_ps")
                    actT = actp.tile([P, KC2, P], F16, tag="actT", name="actT")
                    y_ps = py.tile([P, H], F32, tag="y", name="y_ps")
                    for g in range(G2):
                        w2_t = w2pool.tile([P, 2, H], F16, tag="w2", name="w2_t")
                        nc.sync.dma_start(w2_t, w2p[e, g])
                        for j in range(2):
                            ic = 2 * g + j
                            nc.tensor.transpose(
                                tp_ps[:, ic, :],
                                act[:, ic * P : (ic + 1) * P],
                                ident,
                            )
                            nc.vector.tensor_copy(
                                actT[:, ic, :], tp_ps[:, ic, :]
                            )
                            for nb in range(H // FD):
                                nc.tensor.matmul(
                                    y_ps[:, nb * FD : (nb + 1) * FD],
                                    actT[:, ic, :],
                                    w2_t[:, j, nb * FD : (nb + 1) * FD],
                                    start=(ic == 0),
                                    stop=(ic == KC2 - 1),
                                )
                    for hh in range(4):
                        QH = H // 4
                        y_sb = yp.tile([P, QH], F16, tag="ysb", name="y_sb")
                        nc.vector.tensor_copy(
                            y_sb, y_ps[:, hh * QH : (hh + 1) * QH]
                        )
                        nc.scalar.dma_start(
                            y[
                                e,
                                cb * P : (cb + 1) * P,
                                hh * QH : (hh + 1) * QH,
                            ],
                            y_sb,
                        )
    nc.finalize()
    return nc


def _route(router_logits, top_k):
    """softmax -> top-k -> renormalize; per-expert token lists + weights."""
    lg = np.asarray(router_logits, dtype=np.float64)
    T, num_e = lg.shape
    k = int(np.asarray(top_k))
    p = np.exp(lg - lg.max(axis=-1, keepdims=True))
    p /= p.sum(axis=-1, keepdims=True)
    idx = np.argpartition(-p, k - 1, axis=1)[:, :k]  # [T, k] top-k set
    vals = np.take_along_axis(p, idx, axis=1)
    wts = vals / vals.sum(axis=-1, keepdims=True)
    tok_idx = [[] for _ in range(num_e)]
    tok_w = [[] for _ in range(num_e)]
    for t in range(T):
        for j in range(k):
            tok_idx[idx[t, j]].append(t)
            tok_w[idx[t, j]].append(wts[t, j])
    return tok_idx, tok_w


def kernel(x, router_logits, w1, w2, top_k):
    global LAST_RESULTS
    x = np.asarray(x)
    w1 = np.asarray(w1)
    w2 = np.asarray(w2)
    T = x.shape[0]

    tok_idx, tok_w = _route(router_logits, top_k)
    max_count = max(max(len(ti) for ti in tok_idx), 1)
    C = ((max_count + P - 1) // P) * P

    prog = _programs.get(C)
    if prog is None:
        prog = _programs[C] = _build_program(C)

    KC1 = H // P
    xT16 = np.ascontiguousarray(x.T.astype(np.float16))  # [H, T]
    in_maps = []
    for c in range(N_CORES):
        sl = slice(c * E_LOC, (c + 1) * E_LOC)
        # [E_LOC, H, 2I] -> [E_LOC, G1, P, 2, I2] (pair k-chunks in free dim)
        w1tc = w1[sl].transpose(0, 2, 1).astype(np.float16)
        w1pc = np.ascontiguousarray(
            w1tc.reshape(E_LOC, KC1 // 2, 2, P, I2).transpose(0, 1, 3, 2, 4)
        )
        w2tc = w2[sl].transpose(0, 2, 1).astype(np.float16)  # [E_LOC, I, H]
        w2pc = np.ascontiguousarray(
            w2tc.reshape(E_LOC, I // P // 2, 2, P, H).transpose(0, 1, 3, 2, 4)
        )
        xgc = np.zeros((E_LOC, P, KC1, C), np.float16)
        for el in range(E_LOC):
            ti = tok_idx[c * E_LOC + el]
            if ti:
                # [H, n] -> [KC1, P, n] -> [P, KC1, n]
                xgc[el, :, :, : len(ti)] = (
                    xT16[:, ti].reshape(KC1, P, len(ti)).transpose(1, 0, 2)
                )
        in_maps.append({"w1p": w1pc, "w2p": w2pc, "xg": xgc})

    LAST_RESULTS = bass_utils.run_bass_kernel_spmd(
        prog,
        in_maps,
        core_ids=list(range(N_CORES)),
        trace=TRACE,
        trace_cores=TRACE_CORES,
    )

    out = np.zeros((T, H), dtype=np.float64)
    for c in range(N_CORES):
        yv = LAST_RESULTS.results[c]["y"]  # [E_LOC, C, H] fp16
        for el in range(E_LOC):
            ge = c * E_LOC + el
            ti = tok_idx[ge]
            if ti:
                wv = np.asarray(tok_w[ge], dtype=np.float64)[:, None]
                out[ti] += wv * yv[el][: len(ti)].astype(np.float64)
    return out.astype(x.dtype)



# revision 3
# speedup vs baseline: 1.1711x; 1.1711x over previous
"""MoE expert FFN (CachedKimiExperts) on 8 Trainium2 NeuronCores.

Expert-parallel sharding: core c owns experts [2c, 2c+1].  Routing
(softmax -> top-k -> renormalize) and token gather/scatter run on the
host; each core streams its two experts' weights (pre-transposed,
cast to fp16 on the host) from HBM once and computes

    h   = x_g @ w1[e].T          (gate/up fused, contract over H)
    act = silu(h[:, :I]) * h[:, I:]
    y   = act @ w2[e].T          (contract over I)

for the gathered token block of each expert.  The host applies the
routing weights and scatter-adds the per-expert outputs.

v2 pipeline: w1 is split into two column-halves (each holding gate and
up columns for one half of I) so the down-projection of half h can run
as soon as mm1 for that half finishes -- the per-expert DMA stream is

    xg, w1[h0] x4MiB, w2[ic 0..3] x2MiB, w1[h1] x4MiB, w2[ic 4..7] x2MiB

which keeps the 16 SDMA engines busy end-to-end (no pool-slot WAR
stalls: all of one expert's tiles fit in SBUF simultaneously) and
leaves only ~2 i-chunks of mm2 + the y writeback after the last weight
byte lands.
"""

import os
import sys

import numpy as np

for _p in ("/opt/trn_rl_repo", "/root/.axon_site/_ro/trn_rl_repo"):
    if os.path.isdir(_p) and _p not in sys.path:
        sys.path.append(_p)

import concourse.bass as bass  # noqa: F401  (bass must import before tile)
import concourse.mybir as mybir
import concourse.tile as tile
from concourse import bacc, bass_utils
from concourse.masks import make_identity

N_CORES = 8
E = 16
E_LOC = E // N_CORES  # experts per core
H = 2048  # hidden dim
I = 1024  # expert intermediate dim
I2 = 2 * I  # fused gate+up width
IH = I // 2  # half of the intermediate dim
P = 128  # partitions
FD = 512  # matmul moving free dim (one fp32 PSUM bank)

F16 = mybir.dt.float16
F32 = mybir.dt.float32

TRACE = False
TRACE_CORES = None
LAST_RESULTS = None

_programs = {}


def _build_program(C):
    """Bass/Tile program for one core: E_LOC experts x (C tokens each)."""
    KC1 = H // P  # k-chunks for the gate/up matmul (contract over H)
    G1 = 4  # w1 tile groups per half (4 k-chunks per 1MiB tile)
    KPT = KC1 // G1  # k-chunks per w1 tile
    G2 = 4  # w2 tiles per expert (2 i-chunks per 1MiB tile)
    CB = C // P  # token blocks per expert
    NIC = I // P  # i-chunks for the down matmul
    NICH = NIC // 2  # i-chunks per half

    nc = bacc.Bacc(
        "TRN2", target_bir_lowering=False, debug=False, num_devices=N_CORES
    )
    # w1p[e, h, g, p, j, 0:512]   = gate cols h*512+[0,512) of w1[2c+e].T,
    #                               k-row (4g+j)*128 + p
    # w1p[e, h, g, p, j, 512:1024] = up cols   (same half, same k-row)
    w1p = nc.dram_tensor("w1p", [E_LOC, 2, G1, P, KPT, 2 * IH], F16,
                         kind="ExternalInput")
    # w2p[e, g, p, j, :] = w2[2c+e].T[(2g+j)*128 + p, :]
    w2p = nc.dram_tensor("w2p", [E_LOC, G2, P, 2, H], F16, kind="ExternalInput")
    # xg[e, p, kc, c] = x.T[kc*128 + p, tok_c(e)]  (gathered, padded)
    xg = nc.dram_tensor("xg", [E_LOC, P, KC1, C], F16, kind="ExternalInput")
    y = nc.dram_tensor("y", [E_LOC, C, H], F16, kind="ExternalOutput")

    with tile.TileContext(nc) as tc:
        with (
            tc.tile_pool(name="w1pool", bufs=10) as w1pool,
            tc.tile_pool(name="w2pool", bufs=6) as w2pool,
            tc.tile_pool(name="xp", bufs=2) as xp,
            tc.tile_pool(name="actp", bufs=2) as actp,
            tc.tile_pool(name="yp", bufs=4) as yp,
            tc.tile_pool(name="constp", bufs=1) as constp,
            tc.tile_pool(name="pgu", bufs=4, space="PSUM") as pgu,
            tc.tile_pool(name="py", bufs=1, space="PSUM") as py,
        ):
            ident = constp.tile([P, P], F16, name="ident")
            make_identity(nc, ident)

            # HAM warmup: ~4us of dummy matmuls during the initial DMA wait
            # flips the PE clock gate to 2.4GHz before the real stream
            # starts (idle/cold default is 1.2GHz, needs ~3.4us of activity).
            warm_ps = pgu.tile([P, FD], F32, tag="gu", name="warm_ps")
            for _ in range(40):
                nc.tensor.matmul(
                    warm_ps[:, :P], ident, ident, start=True, stop=True
                )

            for e in range(E_LOC):
                # ---- DMA stream for this expert (issue order == Sync
                # program order): xg, w1 half0, w2 ic0-3, w1 half1, w2 ic4-7
                xg_t = xp.tile([P, KC1, C], F16, tag="xg", name="xg_t")
                hk = KC1 // 2
                nc.sync.dma_start(xg_t[:, :hk, :], xg[e, :, :hk, :])
                nc.sync.dma_start(xg_t[:, hk:, :], xg[e, :, hk:, :])

                w1_t = [[None] * G1 for _ in range(2)]
                w2_t = [None] * G2
                for h in range(2):
                    for g in range(G1):
                        t = w1pool.tile([P, KPT, 2 * IH], F16, tag="w1",
                                        name="w1_t")
                        if e == 0 and h == 0 and g == 0:
                            # split the very first tile so matmuls start
                            # after 512KB instead of 1MB
                            nc.sync.dma_start(
                                t[:, : KPT // 2, :], w1p[e, h, g, :, : KPT // 2, :]
                            )
                            nc.sync.dma_start(
                                t[:, KPT // 2 :, :], w1p[e, h, g, :, KPT // 2 :, :]
                            )
                        else:
                            nc.sync.dma_start(t, w1p[e, h, g])
                        w1_t[h][g] = t
                    for g in (0, 1) if h == 0 else (2, 3):
                        t = w2pool.tile([P, 2, H], F16, tag="w2", name="w2_t")
                        nc.sync.dma_start(t, w2p[e, g])
                        w2_t[g] = t

                for cb in range(CB):
                    cs = slice(cb * P, (cb + 1) * P)
                    y_ps = py.tile([P, H], F32, tag="y", name="y_ps")
                    for h in range(2):
                        # ---- gate/up projection for this half ----
                        gate_ps = pgu.tile([P, IH], F32, tag="gu", name="gate_ps")
                        up_ps = pgu.tile([P, IH], F32, tag="gu", name="up_ps")
                        for g in range(G1):
                            for j in range(KPT):
                                kc = KPT * g + j
                                lhsT = xg_t[:, kc, cs]
                                st = kc == 0
                                sp = kc == KC1 - 1
                                nc.tensor.matmul(
                                    gate_ps,
                                    lhsT,
                                    w1_t[h][g][:, j, :IH],
                                    start=st,
                                    stop=sp,
                                )
                                nc.tensor.matmul(
                                    up_ps,
                                    lhsT,
                                    w1_t[h][g][:, j, IH:],
                                    start=st,
                                    stop=sp,
                                )

                        # ---- act = silu(gate) * up, cast to fp16 ----
                        sg = actp.tile([P, IH], F32, tag="sg", name="sg")
                        act = actp.tile([P, IH], F16, tag="act", name="act")
                        for q in range(2):
                            qs = slice(q * (IH // 2), (q + 1) * (IH // 2))
                            nc.scalar.activation(
                                sg[:, qs],
                                gate_ps[:, qs],
                                mybir.ActivationFunctionType.Silu,
                            )
                            nc.vector.tensor_mul(act[:, qs], sg[:, qs], up_ps[:, qs])

                        # ---- transpose + down projection per i-chunk ----
                        for icl in range(NICH):
                            ic = h * NICH + icl
                            tp_ps = pgu.tile([P, P], F16, tag="gu", name="tp_ps")
                            actT = actp.tile([P, P], F16, tag="actT", name="actT")
                            nc.tensor.transpose(
                                tp_ps, act[:, icl * P : (icl + 1) * P], ident
                            )
                            nc.vector.tensor_copy(actT, tp_ps)
                            wt = w2_t[ic // 2]
                            for nb in range(H // FD):
                                nc.tensor.matmul(
                                    y_ps[:, nb * FD : (nb + 1) * FD],
                                    actT,
                                    wt[:, ic % 2, nb * FD : (nb + 1) * FD],
                                    start=(ic == 0),
                                    stop=(ic == NIC - 1),
                                )

                    for hh in range(4):
                        QH = H // 4
                        y_sb = yp.tile([P, QH], F16, tag="ysb", name="y_sb")
                        nc.vector.tensor_copy(
                            y_sb, y_ps[:, hh * QH : (hh + 1) * QH]
                        )
                        nc.scalar.dma_start(
                            y[e, cs, hh * QH : (hh + 1) * QH],
                            y_sb,
                        )
    nc.finalize()
    return nc


def _route(router_logits, top_k):
    """softmax -> top-k -> renormalize; per-expert token lists + weights."""
    lg = np.asarray(router_logits, dtype=np.float64)
    T, num_e = lg.shape
    k = int(np.asarray(top_k))
    p = np.exp(lg - lg.max(axis=-1, keepdims=True))
    p /= p.sum(axis=-1, keepdims=True)
    idx = np.argpartition(-p, k - 1, axis=1)[:, :k]  # [T, k] top-k set
    vals = np.take_along_axis(p, idx, axis=1)
    wts = vals / vals.sum(axis=-1, keepdims=True)
    tok_idx = [[] for _ in range(num_e)]
    tok_w = [[] for _ in range(num_e)]
    for t in range(T):
        for j in range(k):
            tok_idx[idx[t, j]].append(t)
            tok_w[idx[t, j]].append(wts[t, j])
    return tok_idx, tok_w


def kernel(x, router_logits, w1, w2, top_k):
    global LAST_RESULTS
    x = np.asarray(x)
    w1 = np.asarray(w1)
    w2 = np.asarray(w2)
    T = x.shape[0]

    tok_idx, tok_w = _route(router_logits, top_k)
    max_count = max(max(len(ti) for ti in tok_idx), 1)
    C = ((max_count + P - 1) // P) * P

    prog = _programs.get(C)
    if prog is None:
        prog = _programs[C] = _build_program(C)

    KC1 = H // P
    G1 = 4
    KPT = KC1 // G1
    xT16 = np.ascontiguousarray(x.T.astype(np.float16))  # [H, T]
    in_maps = []
    for c in range(N_CORES):
        sl = slice(c * E_LOC, (c + 1) * E_LOC)
        w1tc = w1[sl].transpose(0, 2, 1).astype(np.float16)  # [E_LOC, H, 2I]
        # [E_LOC, 2, G1, P, KPT, 2*IH]: half h holds gate/up cols
        # [h*IH, (h+1)*IH); k-row (g*KPT + j)*128 + p
        w1pc = np.empty((E_LOC, 2, G1, P, KPT, 2 * IH), np.float16)
        for h in range(2):
            blk = np.concatenate(
                [
                    w1tc[:, :, h * IH : (h + 1) * IH],
                    w1tc[:, :, I + h * IH : I + (h + 1) * IH],
                ],
                axis=2,
            )  # [E_LOC, H, 2*IH]
            w1pc[:, h] = blk.reshape(E_LOC, G1, KPT, P, 2 * IH).transpose(
                0, 1, 3, 2, 4
            )
        w1pc = np.ascontiguousarray(w1pc)
        w2tc = w2[sl].transpose(0, 2, 1).astype(np.float16)  # [E_LOC, I, H]
        w2pc = np.ascontiguousarray(
            w2tc.reshape(E_LOC, I // P // 2, 2, P, H).transpose(0, 1, 3, 2, 4)
        )
        xgc = np.zeros((E_LOC, P, KC1, C), np.float16)
        for el in range(E_LOC):
            ti = tok_idx[c * E_LOC + el]
            if ti:
                # [H, n] -> [KC1, P, n] -> [P, KC1, n]
                xgc[el, :, :, : len(ti)] = (
                    xT16[:, ti].reshape(KC1, P, len(ti)).transpose(1, 0, 2)
                )
        in_maps.append({"w1p": w1pc, "w2p": w2pc, "xg": xgc})

    LAST_RESULTS = bass_utils.run_bass_kernel_spmd(
        prog,
        in_maps,
        core_ids=list(range(N_CORES)),
        trace=TRACE,
        trace_cores=TRACE_CORES,
    )

    out = np.zeros((T, H), dtype=np.float64)
    for c in range(N_CORES):
        yv = LAST_RESULTS.results[c]["y"]  # [E_LOC, C, H] fp16
        for el in range(E_LOC):
            ge = c * E_LOC + el
            ti = tok_idx[ge]
            if ti:
                wv = np.asarray(tok_w[ge], dtype=np.float64)[:, None]
                out[ti] += wv * yv[el][: len(ti)].astype(np.float64)
    return out.astype(x.dtype)
